# revision 9
# baseline (speedup 1.0000x reference)
"""Causal self-attention kernel for 8 Trainium2 NeuronCores.

Problem (hardcoded): x [4, 2048, 768] f32, W [768, 2304] f32, b [2304] f32.
reference: qkv = x@W+b; 8 heads, head_dim 96; causal softmax attention.

Sharding: core c handles batch c//2 and heads 4*(c%2) .. 4*(c%2)+3
(data-parallel over batch x tensor-parallel over heads). Host shards
inputs / gathers outputs around one SPMD NEFF; no device collectives.

Per-core device program (all matmul operands bf16, PSUM f32):
  - projection: qT,kT computed transposed [96, seq] (bias added as
    per-partition scalar), v computed natural [seq, 96] (bias added via a
    K=1 matmul against a ones row), with a ones column appended to v so
    the PV matmul also produces the softmax denominator.
  - attention in S^T layout: S^T[k,q] = kT.T @ qT per 512-wide q block;
    exp (scale folded in) without max subtraction (logits ~N(0,1));
    causal = skip fully-masked 128-chunks + affine_select on diagonal
    chunks. The pr loop is software-pipelined: S(pr+1) is emitted before
    PV(pr) so the PE never waits on the exp.
  - PV in natural layout: o[q,d] += pt[k,q128].T @ va[k,:] per 128-q
    chunk (bf16 runs 1 cycle/row at any free size), so no output
    transposes are needed; denominator comes out as column 96.
"""

import functools
from contextlib import ExitStack

import numpy as np
import ml_dtypes

import concourse.bacc as bacc
import concourse.bass as bass
import concourse.mybir as mybir
import concourse.tile as tile
from concourse.bass_utils import run_bass_kernel_spmd

F32 = mybir.dt.float32
BF16 = mybir.dt.bfloat16

B, N, C, H = 4, 2048, 768, 8
D = C // H            # 96
NCORES = 8
LH = 4                # local heads per core
KC = C // 128         # 6 contraction chunks
NB = N // 512         # 4 seq blocks of 512
OUTC = LH * D         # 384
SCALE = float(1.0 / np.sqrt(np.float32(D)))
UNROLL2 = False


@functools.lru_cache(maxsize=4)
def build(reps=1, use_f32r=True):
    nc = bacc.Bacc("TRN2", target_bir_lowering=False, debug=False,
                   num_devices=NCORES)
    xt_d = nc.dram_tensor("xt", [C, N], BF16, kind="ExternalInput")
    wqk_d = nc.dram_tensor("wqk", [C, 2 * LH * D], BF16, kind="ExternalInput")
    wv_d = nc.dram_tensor("wv", [C, LH * D], BF16, kind="ExternalInput")
    bqk_d = nc.dram_tensor("bqk", [D, 2 * LH], F32, kind="ExternalInput")
    bv_d = nc.dram_tensor("bv", [1, LH * D], BF16, kind="ExternalInput")
    out_d = nc.dram_tensor("out", [N, OUTC], F32, kind="ExternalOutput")

    xt_v = xt_d.ap().rearrange("(kc p) n -> p kc n", p=128)
    wqk_v = wqk_d.ap().rearrange("(kc p) m -> p kc m", p=128)
    wv_v = wv_d.ap().rearrange("(kc p) m -> p kc m", p=128)
    out_v = out_d.ap().rearrange("(qq t p) c -> qq p t c", t=4, p=128)

    with tile.TileContext(nc) as tc, ExitStack() as ctx:
        const = ctx.enter_context(tc.tile_pool(name="const", bufs=1))
        wpool = ctx.enter_context(tc.tile_pool(name="w", bufs=1))
        xpool = ctx.enter_context(tc.tile_pool(name="x", bufs=3))
        qkpool = ctx.enter_context(tc.tile_pool(name="qk", bufs=NB))
        vpool = ctx.enter_context(tc.tile_pool(name="vaug", bufs=NB))
        ppool = ctx.enter_context(tc.tile_pool(name="p", bufs=6))
        rpool = ctx.enter_context(tc.tile_pool(name="r", bufs=4))
        spool = ctx.enter_context(tc.tile_pool(name="stage", bufs=3))
        ps_proj = ctx.enter_context(
            tc.tile_pool(name="ps_proj", bufs=2, space="PSUM"))
        ps_s = ctx.enter_context(
            tc.tile_pool(name="ps_s", bufs=2, space="PSUM"))
        ps_o = ctx.enter_context(
            tc.tile_pool(name="ps_o", bufs=2, space="PSUM"))

        # one-time constants
        ones_r = const.tile([1, 128], BF16)
        nc.gpsimd.memset(ones_r[:], 1.0)
        vones = const.tile([128, 4, LH, 1], BF16)
        nc.gpsimd.memset(vones[:], 1.0)

        wqk_sb = wpool.tile([128, KC, 2 * LH * D], BF16, tag="wqk")
        wv_sb = wpool.tile([128, KC, LH * D], BF16, tag="wv")
        nc.sync.dma_start(wqk_sb[:], wqk_v[:])
        nc.sync.dma_start(wv_sb[:], wv_v[:])
        bqk_sb = wpool.tile([D, 2 * LH], F32, tag="bqk")
        nc.sync.dma_start(bqk_sb[:], bqk_d.ap())
        bv_sb = wpool.tile([1, LH * D], BF16, tag="bv")
        nc.sync.dma_start(bv_sb[:], bv_d.ap())

        def body():
            qk_tiles = [None] * NB
            va_tiles = [None] * NB
            xts = {}

            def issue_x_dma(nb):
                xt_sb = xpool.tile([128, KC, 512], BF16, tag="xt")
                nc.sync.dma_start(
                    xt_sb[:], xt_v[:, :, nb * 512:(nb + 1) * 512])
                xts[nb] = xt_sb

            def make_proj_ops(nb):
                """Projection of seq block nb as a list of micro-ops (each
                emits ~1 instruction) for interleaving into attention.
                The x DMA for block nb must already be issued."""
                st = {}
                ops = []

                def op_va_init():
                    va = vpool.tile([128, 4, LH, D + 1], BF16, tag="va")
                    nc.vector.tensor_copy(va[:, :, :, D:D + 1], vones[:])
                    va_tiles[nb] = va
                ops.append(op_va_init)
                for mt in range(4):
                    def op_vbias(mt=mt):
                        vps = ps_proj.tile([128, 512], F32, tag="proj")
                        nc.tensor.matmul(
                            vps[:, 0:LH * D], ones_r[:, :], bv_sb[:, :],
                            start=True, stop=False)
                        st['vps'] = vps
                    ops.append(op_vbias)
                    for kc in range(KC):
                        def op_vmm(mt=mt, kc=kc):
                            nc.tensor.matmul(
                                st['vps'][:, 0:LH * D],
                                xts[nb][:, kc, mt * 128:(mt + 1) * 128],
                                wv_sb[:, kc, :],
                                start=False, stop=(kc == KC - 1))
                        ops.append(op_vmm)

                    def op_vcopy(mt=mt):
                        nc.vector.tensor_copy(
                            va_tiles[nb][:, mt, :, 0:D],
                            st['vps'][:, 0:LH * D].rearrange(
                                "p (h d) -> p h d", h=LH))
                    ops.append(op_vcopy)

                def op_qk_alloc():
                    qk_tiles[nb] = [
                        qkpool.tile([D, 2, 512], BF16, tag=f"qk{hh}")
                        for hh in range(LH)]
                ops.append(op_qk_alloc)
                for hh in range(LH):
                    for t in range(2):
                        m = 2 * hh + t
                        for kc in range(KC):
                            def op_qkmm(hh=hh, t=t, m=m, kc=kc):
                                if kc == 0:
                                    st['qps'] = ps_proj.tile(
                                        [128, 512], F32, tag="proj")
                                nc.tensor.matmul(
                                    st['qps'][0:D, :],
                                    wqk_sb[:, kc, m * D:(m + 1) * D],
                                    xts[nb][:, kc, :],
                                    start=(kc == 0), stop=(kc == KC - 1))
                            ops.append(op_qkmm)

                        def op_bias(hh=hh, t=t, m=m):
                            nc.vector.tensor_scalar_add(
                                qk_tiles[nb][hh][:, t, :],
                                st['qps'][0:D, :], bqk_sb[:, m:m + 1])
                        ops.append(op_bias)
                return ops

            def attention(Q, h, popper=None):
                """Attention for q block Q, local head h (pipelined).
                popper() is called once per pr iteration to interleave
                projection micro-ops of a later block into the PE stream."""
                njc = 4 * Q + 4      # k chunks of 128
                nprs = njc // 2      # pairs

                def emit_S(pr):
                    sps = ps_s.tile([128, 1024], F32, tag="s")
                    info = []
                    for idx in range(2):
                        j = 2 * pr + idx
                        qoff = max(512 * Q, 128 * j)
                        width = 512 * (Q + 1) - qoff
                        info.append((j, qoff, width))
                        nc.tensor.matmul(
                            sps[:, idx * 512:idx * 512 + width],
                            qk_tiles[j // 4][h][
                                :, 1, (j % 4) * 128:(j % 4) * 128 + 128],
                            qk_tiles[Q][h][
                                :, 0, qoff - 512 * Q:qoff - 512 * Q + width],
                            start=True, stop=True)
                    return sps, info

                # one PSUM bank; all 4 q-chunks form ONE accumulation group
                # (start zeroes the full 2KB zero region, so later chunks
                # accumulate onto pending-zeros; stop on the last matmul)
                ops = ps_o.tile([128, 4, 128], F32, tag="o")
                prev = emit_S(0)
                for pr in range(nprs):
                    sps, info = prev
                    if pr + 1 < nprs:
                        prev = emit_S(pr + 1)
                    if popper is not None:
                        popper()
                    pt = ppool.tile([128, 1024], BF16, tag="p")
                    w0, w1 = info[0][2], info[1][2]
                    if w0 == 512:
                        nc.scalar.activation(
                            pt[:, 0:512 + w1], sps[:, 0:512 + w1],
                            mybir.ActivationFunctionType.Exp, scale=SCALE)
                    else:
                        nc.scalar.activation(
                            pt[:, 0:w0], sps[:, 0:w0],
                            mybir.ActivationFunctionType.Exp, scale=SCALE)
                        nc.scalar.activation(
                            pt[:, 512:512 + w1], sps[:, 512:512 + w1],
                            mybir.ActivationFunctionType.Exp, scale=SCALE)
                    for idx, (j, qoff, width) in enumerate(info):
                        if j >= 4 * Q:  # diagonal chunk: causal mask
                            nc.gpsimd.affine_select(
                                out=pt[:, idx * 512:idx * 512 + 128],
                                in_=pt[:, idx * 512:idx * 512 + 128],
                                compare_op=mybir.AluOpType.is_ge,
                                fill=0.0, base=0, pattern=[[1, 128]],
                                channel_multiplier=-1)
                        va_j = va_tiles[j // 4][:, j % 4, h, :]
                        for t in range(4):
                            if j > 4 * Q + t:
                                continue  # fully masked 128-chunk
                            qg0 = 512 * Q + 128 * t
                            col0 = idx * 512 + (qg0 - qoff)
                            nc.tensor.matmul(
                                ops[:, t, 0:D + 1],
                                pt[:, col0:col0 + 128],
                                va_j,
                                start=(j == 0 and t == 0),
                                stop=(j == 4 * Q + 3 and t == 3))

                # normalize + store
                rr = rpool.tile([128, 4], F32, tag="r")
                nc.vector.reciprocal(rr[:], ops[:, :, D])
                stage = spool.tile([128, 4, D], F32, tag="stage")
                for t in range(4):
                    nc.vector.tensor_scalar_mul(
                        stage[:, t, :], ops[:, t, 0:D], rr[:, t:t + 1])
                nc.sync.dma_start(
                    out_v[Q, :, :, h * D:(h + 1) * D], stage[:])

            # preamble: x DMAs for blocks 0,1 + full projection of block 0
            issue_x_dma(0)
            issue_x_dma(1)
            for op in make_proj_ops(0):
                op()

            for Q in range(NB):
                if Q + 2 < NB:
                    issue_x_dma(Q + 2)
                queue = make_proj_ops(Q + 1) if Q + 1 < NB else []
                qi = [0]
                points_left = [LH * (2 * Q + 2)]

                def popper(queue=queue, qi=qi, points_left=points_left):
                    rem = len(queue) - qi[0]
                    if rem > 0:
                        k = -(-rem // max(points_left[0], 1))
                        for _ in range(min(k, rem)):
                            queue[qi[0]]()
                            qi[0] += 1
                    points_left[0] -= 1

                for h in range(LH):
                    attention(Q, h, popper)
                while qi[0] < len(queue):  # drain leftovers
                    queue[qi[0]]()
                    qi[0] += 1

        if reps == 1:
            body()
        else:
            with tc.For_i(0, reps, 1):
                body()
                if UNROLL2:
                    body()

    nc.compile()
    return nc


def shard_inputs(x, W, b, use_f32r=True):
    """Full inputs -> per-core in_maps (bf16 matmul operands)."""
    x = np.asarray(x, dtype=np.float32)
    W = np.asarray(W, dtype=np.float32)
    b = np.asarray(b, dtype=np.float32)
    bf = lambda a: np.ascontiguousarray(
        np.asarray(a, dtype=np.float32)).astype(ml_dtypes.bfloat16)
    in_maps = []
    for c in range(NCORES):
        bc, g = divmod(c, 2)
        h0 = g * LH
        qcols = [W[:, 0 * C + (h0 + h) * D:0 * C + (h0 + h + 1) * D]
                 for h in range(LH)]
        kcols = [W[:, 1 * C + (h0 + h) * D:1 * C + (h0 + h + 1) * D]
                 for h in range(LH)]
        vcols = [W[:, 2 * C + (h0 + h) * D:2 * C + (h0 + h + 1) * D]
                 for h in range(LH)]
        wqk = np.concatenate(
            [m for h in range(LH) for m in (qcols[h], kcols[h])], axis=1)
        wv = np.concatenate(vcols, axis=1)
        bqk = np.stack(
            [b[t * C + (h0 + h) * D:t * C + (h0 + h + 1) * D]
             for h in range(LH) for t in (0, 1)], axis=1)
        bv = np.concatenate(
            [b[2 * C + (h0 + h) * D:2 * C + (h0 + h + 1) * D]
             for h in range(LH)])[None, :]
        in_maps.append({
            "xt": bf(x[bc].T),
            "wqk": bf(wqk),
            "wv": bf(wv),
            "bqk": np.ascontiguousarray(bqk),
            "bv": bf(bv),
        })
    return in_maps


def gather_outputs(results):
    """Per-core results -> full [B, N, C] output."""
    out = np.empty((B, N, C), dtype=np.float32)
    for c in range(NCORES):
        bc, g = divmod(c, 2)
        out[bc, :, g * OUTC:(g + 1) * OUTC] = results[c]["out"]
    return out


def kernel(x, W, b):
    nc = build(reps=1)
    in_maps = shard_inputs(x, W, b)
    res = run_bass_kernel_spmd(nc, in_maps, core_ids=list(range(NCORES)))
    return gather_outputs(res.results)


# revision 20
# speedup vs baseline: 1.2118x; 1.2118x over previous
"""Causal self-attention kernel for 8 Trainium2 NeuronCores.

Problem (hardcoded): x [4, 2048, 768] f32, W [768, 2304] f32, b [2304] f32.
reference: qkv = x@W+b; 8 heads, head_dim 96; causal softmax attention.

Sharding: core c handles batch c//2 and heads 4*(c%2) .. 4*(c%2)+3
(data-parallel over batch x tensor-parallel over heads). Host shards
inputs / gathers outputs around one SPMD NEFF; no device collectives.

Per-core device program (all matmul operands bf16, PSUM f32):
  - projection: qT,kT computed transposed [96, seq] (bias added as
    per-partition scalar), v computed natural [seq, 96] (bias added via a
    K=1 matmul against a ones row), with a ones column appended to v so
    the PV matmul also produces the softmax denominator.
  - attention in S^T layout: S^T[k,q] = kT.T @ qT per 512-wide q block;
    exp (scale folded in) without max subtraction (logits ~N(0,1));
    causal = skip fully-masked 128-chunks + affine_select on diagonal
    chunks. The pr loop is software-pipelined: S(pr+1) is emitted before
    PV(pr) so the PE never waits on the exp.
  - PV in natural layout: o[q,d] += pt[k,q128].T @ va[k,:] per 128-q
    chunk (bf16 runs 1 cycle/row at any free size), so no output
    transposes are needed; denominator comes out as column 96.
"""

import functools
from contextlib import ExitStack

import numpy as np
import ml_dtypes

import concourse.bacc as bacc
import concourse.bass as bass
import concourse.mybir as mybir
import concourse.tile as tile
from concourse.bass_utils import run_bass_kernel_spmd

F32 = mybir.dt.float32
BF16 = mybir.dt.bfloat16

B, N, C, H = 4, 2048, 768, 8
D = C // H            # 96
NCORES = 8
LH = 4                # local heads per core
KC = C // 128         # 6 contraction chunks
NB = N // 512         # 4 seq blocks of 512
OUTC = LH * D         # 384
SCALE = float(1.0 / np.sqrt(np.float32(D)))
UNROLL2 = False


@functools.lru_cache(maxsize=4)
def build(reps=1, use_f32r=True, py_unroll=False):
    nc = bacc.Bacc("TRN2", target_bir_lowering=False, debug=False,
                   num_devices=NCORES)
    xt_d = nc.dram_tensor("xt", [C, N], BF16, kind="ExternalInput")
    wqk_d = nc.dram_tensor("wqk", [C, 2 * LH * D], BF16, kind="ExternalInput")
    wv_d = nc.dram_tensor("wv", [C, LH * D], BF16, kind="ExternalInput")
    bqk_d = nc.dram_tensor("bqk", [D, 2 * LH], F32, kind="ExternalInput")
    bv_d = nc.dram_tensor("bv", [1, LH * D], BF16, kind="ExternalInput")
    out_d = nc.dram_tensor("out", [N, OUTC], F32, kind="ExternalOutput")

    xt_v = xt_d.ap().rearrange("(kc p) n -> p kc n", p=128)
    wqk_v = wqk_d.ap().rearrange("(kc p) m -> p kc m", p=128)
    wv_v = wv_d.ap().rearrange("(kc p) m -> p kc m", p=128)
    out_v = out_d.ap().rearrange("(qq t p) c -> qq p t c", t=4, p=128)

    with tile.TileContext(nc) as tc, ExitStack() as ctx:
        const = ctx.enter_context(tc.tile_pool(name="const", bufs=1))
        wpool = ctx.enter_context(tc.tile_pool(name="w", bufs=1))
        xpool = ctx.enter_context(tc.tile_pool(name="x", bufs=2))
        qkpool = ctx.enter_context(tc.tile_pool(name="qk", bufs=3))
        vpool = ctx.enter_context(tc.tile_pool(name="vaug", bufs=3))
        ppool = ctx.enter_context(tc.tile_pool(name="p", bufs=6))
        rpool = ctx.enter_context(tc.tile_pool(name="r", bufs=4))
        spool = ctx.enter_context(tc.tile_pool(name="stage", bufs=3))
        ps_proj = ctx.enter_context(
            tc.tile_pool(name="ps_proj", bufs=2, space="PSUM"))
        ps_s = ctx.enter_context(
            tc.tile_pool(name="ps_s", bufs=2, space="PSUM"))
        ps_o = ctx.enter_context(
            tc.tile_pool(name="ps_o", bufs=2, space="PSUM"))

        # one-time constants
        ones_r = const.tile([1, 128], BF16)
        nc.gpsimd.memset(ones_r[:], 1.0)
        vones = const.tile([128, 4, LH, 1], BF16)
        nc.gpsimd.memset(vones[:], 1.0)

        wqk_sb = wpool.tile([128, KC, 2 * LH * D], BF16, tag="wqk")
        wv_sb = wpool.tile([128, KC, LH * D], BF16, tag="wv")
        nc.sync.dma_start(wqk_sb[:], wqk_v[:])
        nc.sync.dma_start(wv_sb[:], wv_v[:])
        bqk_sb = wpool.tile([D, 2 * LH], F32, tag="bqk")
        nc.sync.dma_start(bqk_sb[:], bqk_d.ap())
        bv_sb = wpool.tile([1, LH * D], BF16, tag="bv")
        nc.sync.dma_start(bv_sb[:], bv_d.ap())

        # v bias broadcast to all partitions once (outside the loop):
        # vbias_sb[p, h, d] = b_v[h, d], added during the va PSUM->SBUF copy
        bias_ps = ps_proj.tile([128, 512], F32, tag="proj", name="bias_ps")
        nc.tensor.matmul(bias_ps[:, 0:LH * D], ones_r[:, :], bv_sb[:, :],
                         start=True, stop=True)
        vbias_sb = wpool.tile([128, LH, D], F32, tag="vbias")
        nc.vector.tensor_copy(
            vbias_sb[:],
            bias_ps[:, 0:LH * D].rearrange("p (h d) -> p h d", h=LH))

        # Cross-body carry: block-0 qk/va tiles and the block-1 x tile are
        # produced during the PREVIOUS body's attention(3) phase (rotated
        # schedule). With UNROLL2 the two unrolled bodies ping-pong between
        # 2 carry buffers, giving a period-2 cycle that is consistent with
        # the fixed addresses a For_i hardware loop replays.
        carry = {}
        qk_tiles = [None] * NB
        va_tiles = [None] * NB
        xts = {}

        def issue_x_dma(nb, tag):
            xt_sb = xpool.tile([128, KC, 512], BF16, tag=tag, name=tag)
            nc.sync.dma_start(
                xt_sb[:], xt_v[:, :, nb * 512:(nb + 1) * 512])
            xts[nb] = xt_sb
            if nb == 1:
                carry['x1'] = xt_sb

        def make_proj_ops(nb, to_carry=False):
            """Projection of seq block nb as a list of micro-ops (each
            emits ~1 instruction) for interleaving into attention.
            The x DMA for block nb must already be issued. With to_carry,
            results go to the carry slots (consumed by the NEXT body)
            instead of the live qk_tiles/va_tiles lists."""
            st = {}
            ops = []
            sfx = "c" if to_carry else ""

            def op_va_init():
                va = vpool.tile([128, 4, LH, D + 1], BF16,
                                tag="va" + sfx, name="va" + sfx,
                                bufs=2 if to_carry else None)
                nc.vector.tensor_copy(va[:, :, :, D:D + 1], vones[:])
                st['va'] = va
                if to_carry:
                    carry['va0'] = va
                else:
                    va_tiles[nb] = va
            ops.append(op_va_init)
            for mt in range(4):
                for kc in range(KC):
                    def op_vmm(mt=mt, kc=kc):
                        if kc == 0:
                            st['vps'] = ps_proj.tile(
                                [128, 512], F32, tag="proj", name="vps")
                        nc.tensor.matmul(
                            st['vps'][:, 0:LH * D],
                            xts[nb][:, kc, mt * 128:(mt + 1) * 128],
                            wv_sb[:, kc, :],
                            start=(kc == 0), stop=(kc == KC - 1))
                    ops.append(op_vmm)

                def op_vcopy(mt=mt):
                    nc.vector.tensor_add(
                        st['va'][:, mt, :, 0:D],
                        st['vps'][:, 0:LH * D].rearrange(
                            "p (h d) -> p h d", h=LH),
                        vbias_sb[:])
                ops.append(op_vcopy)

            def op_qk_alloc():
                qks = [
                    qkpool.tile([D, 2, 512], BF16, tag=f"qk{hh}" + sfx,
                                name=f"qk{hh}" + sfx,
                                bufs=2 if to_carry else None)
                    for hh in range(LH)]
                st['qk'] = qks
                if to_carry:
                    carry['qk0'] = qks
                else:
                    qk_tiles[nb] = qks
            ops.append(op_qk_alloc)
            for hh in range(LH):
                for t in range(2):
                    m = 2 * hh + t
                    for kc in range(KC):
                        def op_qkmm(hh=hh, t=t, m=m, kc=kc):
                            if kc == 0:
                                st['qps'] = ps_proj.tile(
                                    [128, 512], F32, tag="proj",
                                    name="qps")
                            nc.tensor.matmul(
                                st['qps'][0:D, :],
                                wqk_sb[:, kc, m * D:(m + 1) * D],
                                xts[nb][:, kc, :],
                                start=(kc == 0), stop=(kc == KC - 1))
                        ops.append(op_qkmm)

                    def op_bias(hh=hh, t=t, m=m):
                        nc.vector.tensor_scalar_add(
                            st['qk'][hh][:, t, :],
                            st['qps'][0:D, :], bqk_sb[:, m:m + 1])
                    ops.append(op_bias)
            return ops

        def body():

            def attention(Q, h, popper=None):
                """Attention for q block Q, local head h (pipelined).
                popper() is called once per pr iteration to interleave
                projection micro-ops of a later block into the PE stream."""
                njc = 4 * Q + 4      # k chunks of 128
                nprs = njc // 2      # pairs

                def emit_S(pr):
                    sps = ps_s.tile([128, 1024], F32, tag="s")
                    info = []
                    for idx in range(2):
                        j = 2 * pr + idx
                        qoff = max(512 * Q, 128 * j)
                        width = 512 * (Q + 1) - qoff
                        info.append((j, qoff, width))
                        nc.tensor.matmul(
                            sps[:, idx * 512:idx * 512 + width],
                            qk_tiles[j // 4][h][
                                :, 1, (j % 4) * 128:(j % 4) * 128 + 128],
                            qk_tiles[Q][h][
                                :, 0, qoff - 512 * Q:qoff - 512 * Q + width],
                            start=True, stop=True)
                    return sps, info

                # one PSUM bank; all 4 q-chunks form ONE accumulation group
                # (start zeroes the full 2KB zero region, so later chunks
                # accumulate onto pending-zeros; stop on the last matmul)
                ops = ps_o.tile([128, 4, 128], F32, tag="o")
                prev = emit_S(0)
                for pr in range(nprs):
                    sps, info = prev
                    if pr + 1 < nprs:
                        prev = emit_S(pr + 1)
                    if popper is not None:
                        popper()
                    pt = ppool.tile([128, 1024], BF16, tag="p")
                    w0, w1 = info[0][2], info[1][2]
                    if w0 == 512:
                        nc.scalar.activation(
                            pt[:, 0:512 + w1], sps[:, 0:512 + w1],
                            mybir.ActivationFunctionType.Exp, scale=SCALE)
                    else:
                        nc.scalar.activation(
                            pt[:, 0:w0], sps[:, 0:w0],
                            mybir.ActivationFunctionType.Exp, scale=SCALE)
                        nc.scalar.activation(
                            pt[:, 512:512 + w1], sps[:, 512:512 + w1],
                            mybir.ActivationFunctionType.Exp, scale=SCALE)
                    for idx, (j, qoff, width) in enumerate(info):
                        if j >= 4 * Q:  # diagonal chunk: causal mask
                            nc.gpsimd.affine_select(
                                out=pt[:, idx * 512:idx * 512 + 128],
                                in_=pt[:, idx * 512:idx * 512 + 128],
                                compare_op=mybir.AluOpType.is_ge,
                                fill=0.0, base=0, pattern=[[1, 128]],
                                channel_multiplier=-1)
                        va_j = va_tiles[j // 4][:, j % 4, h, :]
                        for t in range(4):
                            if j > 4 * Q + t:
                                continue  # fully masked 128-chunk
                            qg0 = 512 * Q + 128 * t
                            col0 = idx * 512 + (qg0 - qoff)
                            nc.tensor.matmul(
                                ops[:, t, 0:D + 1],
                                pt[:, col0:col0 + 128],
                                va_j,
                                start=(j == 0 and t == 0),
                                stop=(j == 4 * Q + 3 and t == 3))

                # normalize + store
                rr = rpool.tile([128, 4], F32, tag="r")
                nc.vector.reciprocal(rr[:], ops[:, :, D])
                stage = spool.tile([128, 4, D], F32, tag="stage")
                for t in range(4):
                    nc.vector.tensor_scalar_mul(
                        stage[:, t, :], ops[:, t, 0:D], rr[:, t:t + 1])
                nc.sync.dma_start(
                    out_v[Q, :, :, h * D:(h + 1) * D], stage[:])

            # rotated schedule: block 0's projection was done by the
            # previous body (or the preamble); attention(Q) interleaves
            # the projection of block (Q+1) % NB, with Q=3 producing the
            # NEXT body's block 0 into the carry slots.
            qk_tiles[0] = carry['qk0']
            va_tiles[0] = carry['va0']
            xts[1] = carry['x1']

            for Q in range(NB):
                if Q == 0:
                    issue_x_dma(2, "xt")
                elif Q == 1:
                    issue_x_dma(3, "xt")
                elif Q == 2:
                    issue_x_dma(0, "x0c")   # next body's block 0
                else:
                    issue_x_dma(1, "x1c")   # next body's block 1
                queue = make_proj_ops((Q + 1) % NB, to_carry=(Q == 3))
                qi = [0]
                points_left = [LH * (2 * Q + 2)]

                def popper(queue=queue, qi=qi, points_left=points_left):
                    rem = len(queue) - qi[0]
                    if rem > 0:
                        k = -(-rem // max(points_left[0], 1))
                        for _ in range(min(k, rem)):
                            queue[qi[0]]()
                            qi[0] += 1
                    points_left[0] -= 1

                for h in range(LH):
                    attention(Q, h, popper)
                while qi[0] < len(queue):  # drain leftovers
                    queue[qi[0]]()
                    qi[0] += 1

        # preamble: x DMAs for blocks 0,1 + full projection of block 0
        issue_x_dma(0, "x0c")
        issue_x_dma(1, "x1c")
        for op in make_proj_ops(0, to_carry=True):
            op()

        if reps == 1:
            body()
        elif py_unroll:
            for _ in range(reps):
                body()
                if UNROLL2:
                    body()
        else:
            with tc.For_i(0, reps, 1):
                body()
                if UNROLL2:
                    body()

    nc.compile()
    return nc


def shard_inputs(x, W, b, use_f32r=True):
    """Full inputs -> per-core in_maps (bf16 matmul operands)."""
    x = np.asarray(x, dtype=np.float32)
    W = np.asarray(W, dtype=np.float32)
    b = np.asarray(b, dtype=np.float32)
    bf = lambda a: np.ascontiguousarray(
        np.asarray(a, dtype=np.float32)).astype(ml_dtypes.bfloat16)
    in_maps = []
    for c in range(NCORES):
        bc, g = divmod(c, 2)
        h0 = g * LH
        qcols = [W[:, 0 * C + (h0 + h) * D:0 * C + (h0 + h + 1) * D]
                 for h in range(LH)]
        kcols = [W[:, 1 * C + (h0 + h) * D:1 * C + (h0 + h + 1) * D]
                 for h in range(LH)]
        vcols = [W[:, 2 * C + (h0 + h) * D:2 * C + (h0 + h + 1) * D]
                 for h in range(LH)]
        wqk = np.concatenate(
            [m for h in range(LH) for m in (qcols[h], kcols[h])], axis=1)
        wv = np.concatenate(vcols, axis=1)
        bqk = np.stack(
            [b[t * C + (h0 + h) * D:t * C + (h0 + h + 1) * D]
             for h in range(LH) for t in (0, 1)], axis=1)
        bv = np.concatenate(
            [b[2 * C + (h0 + h) * D:2 * C + (h0 + h + 1) * D]
             for h in range(LH)])[None, :]
        in_maps.append({
            "xt": bf(x[bc].T),
            "wqk": bf(wqk),
            "wv": bf(wv),
            "bqk": np.ascontiguousarray(bqk),
            "bv": bf(bv),
        })
    return in_maps


def gather_outputs(results):
    """Per-core results -> full [B, N, C] output."""
    out = np.empty((B, N, C), dtype=np.float32)
    for c in range(NCORES):
        bc, g = divmod(c, 2)
        out[bc, :, g * OUTC:(g + 1) * OUTC] = results[c]["out"]
    return out


def kernel(x, W, b):
    nc = build(reps=1)
    in_maps = shard_inputs(x, W, b)
    res = run_bass_kernel_spmd(nc, in_maps, core_ids=list(range(NCORES)))
    return gather_outputs(res.results)
